# revision 24
# baseline (speedup 1.0000x reference)
"""Multi-head causal self-attention for TRN2, 8 NeuronCores.

Sharding: core i handles (batch b = i//2, head-group g = i%2); each head-group
is 8 of the 16 heads.  Everything on-device is computed in "transposed" space
(features on partitions, positions on the free dim) so no transposes are
needed.  Single fused pipeline per query block j (no separate phases):

  QKV(j+1) and proj(j-1) matmuls are interleaved into attention(j) so the PE
  never idles while the scalar engine chews through the softmax exps.

  scores:  S^T(h) = kT.T @ qT per head, K=64.  The two heads of a pair are
           packed into partitions 0:64 / 64:128 of one kT/qT tile and issued
           as two row-tiled matmuls (tile_position (0,0)/(64,0)) that run
           CONCURRENTLY in the PE array -> ~2x score throughput.
  softmax: one exp per (chunk, pair) over [128, <=1024] PSUM; on diagonal
           chunks the column range is restricted to the causally live region
           and only the 128-wide triangle gets a mask multiply.
  PV:      stationary operand is [V_h (64 cols) | ones (64 cols)] via a
           strided AP, so PSUM rows 0:63 accumulate O^T and rows 64:127 all
           accumulate the softmax denominator -- already broadcast.  The
           normalize is then just reciprocal_approx_fast + tensor_tensor
           multiply on DVE (no PE broadcast matmul, no single-lane ops).
  proj:    y^T.T @ W_proj in bf16, partial outputs summed on host.

All inputs staged as bf16 on the host (halves DMA, removes all on-device
weight/x casts).  b_qkv handled via DVE per-partition bias add (Q/K) and a
K=1 ones matmul (V).  b_proj added on host.
"""

import numpy as np
import ml_dtypes
from collections import deque
from contextlib import ExitStack

import concourse.bass as bass
import concourse.mybir as mybir
import concourse.tile as tile
from concourse import bacc
from concourse.bass_utils import run_bass_kernel_spmd

B, T, D, H = 4, 2048, 1024, 16
DK = 64            # head dim
HL = 8             # heads per core
DL = HL * DK       # 512 local head dims per core
N_CORES = 8

F32 = mybir.dt.float32
F32R = mybir.dt.float32r
BF16 = mybir.dt.bfloat16
EXP = mybir.ActivationFunctionType.Exp

TQ = 512           # query block size
TKC = 128          # key chunk size
NQB = T // TQ      # 4
NKC = T // TKC     # 16
NDCH = D // 128    # 8 contraction chunks over D
_CACHE = {}


def _build(causal: bool, vbias_zero: bool = True):
    nc = bacc.Bacc("TRN2", target_bir_lowering=False, debug=False,
                   num_devices=N_CORES)
    xT_d = nc.dram_tensor("xT", [D, T], BF16, kind="ExternalInput").ap()
    wqk_d = nc.dram_tensor("wqk", [D, 2 * DL], BF16, kind="ExternalInput").ap()
    wv_d = nc.dram_tensor("wv", [D, DL], BF16, kind="ExternalInput").ap()
    bqk_d = nc.dram_tensor("bqk", [128, 8], F32, kind="ExternalInput").ap()
    bv_d = nc.dram_tensor("bv", [1, DL], F32, kind="ExternalInput").ap()
    wp_d = nc.dram_tensor("wproj", [DL, D], BF16, kind="ExternalInput").ap()
    maskt_d = nc.dram_tensor("maskt", [TKC, 2 * TKC], BF16,
                             kind="ExternalInput").ap()
    out_d = nc.dram_tensor("out", [T, D], F32, kind="ExternalOutput").ap()
    sink_d = nc.dram_tensor("sink", [128, 32], F32, kind="ExternalOutput").ap()

    with tile.TileContext(nc) as tc, ExitStack() as top:
        persist = top.enter_context(tc.tile_pool(name="persist", bufs=1))
        xpool = top.enter_context(tc.tile_pool(name="xpool", bufs=4))
        ppool = top.enter_context(tc.tile_pool(name="ppool", bufs=6))
        rcpool = top.enter_context(tc.tile_pool(name="rcpool", bufs=2))
        otpool = top.enter_context(tc.tile_pool(name="otpool", bufs=2))
        ps_s = top.enter_context(tc.tile_pool(name="ps_s", bufs=2, space="PSUM"))
        ps_o = top.enter_context(tc.tile_pool(name="ps_o", bufs=3, space="PSUM"))
        ps_m = top.enter_context(tc.tile_pool(name="ps_m", bufs=1, space="PSUM"))

        # ---------------- persistent SBUF ----------------
        wqk_sb = persist.tile([128, NDCH, 2 * DL], BF16, name="wqk_sb")
        wv_sb = persist.tile([128, NDCH, DL], BF16, name="wv_sb")
        wp_sb = persist.tile([128, 4, D], BF16, name="wp_sb")
        qT = [persist.tile([128, T], BF16, name=f"qT{i}") for i in range(4)]
        kT = [persist.tile([128, T], BF16, name=f"kT{i}") for i in range(4)]
        # per head h: [V_h (64 cols) | ones (64 cols)] -> PV with this as
        # stationary operand accumulates O^T in PSUM rows 0:63 and the
        # broadcast softmax denominator in rows 64:127.
        vs = [persist.tile([128, HL, 128], BF16, name=f"vs{t}")
              for t in range(NKC)]
        yT = [persist.tile([128, T], BF16, name=f"yT{i}") for i in range(4)]
        bias_sb = persist.tile([128, 8], F32, name="bias_sb")
        if not vbias_zero:
            bv_f = persist.tile([1, DL], F32, name="bv_f")
            bv_r = persist.tile([1, DL], BF16, name="bv_r")
            ones_r = persist.tile([1, 128], BF16, name="ones_r")
        warm = persist.tile([128, TQ], BF16, name="warm")
        scratch = persist.tile([128, 32], F32, name="scratch")
        maskt = persist.tile([TKC, 2, TKC], BF16, name="maskt")

        # PE warmup (keep HAM un-throttled until real matmuls arrive) and
        # early exp table load, while the first DMAs are in flight.
        nc.vector.memset(warm[:], 1.0)
        nc.scalar.activation(scratch[:, 0:16], warm[:, 0:16], EXP, scale=0.125)
        NWARM = 30
        ps_w = ps_m.tile([128, TQ], F32, tag="psm", name="warmps")
        for w in range(NWARM):
            nc.tensor.matmul(ps_w[:], warm[:, 0:128], warm[:],
                             start=(w == 0), stop=(w == NWARM - 1))
        nc.vector.tensor_copy(scratch[:, 16:32], ps_w[:, 0:16])
        nc.gpsimd.dma_start(sink_d, scratch[:])

        if not vbias_zero:
            nc.vector.memset(ones_r[:], 1.0)
        for t in range(NKC):
            nc.vector.memset(vs[t][:, :, 64:128], 1.0)

        # ---------------- DMAs ----------------
        # weight/x loads split across idle queues so QKV(0) can start early
        wqk_src = wqk_d.rearrange("(dd p) m -> p dd m", p=128)
        xsrc = xT_d.rearrange("(dd p) t -> p dd t", p=128)
        xall = [None] * NQB

        def load_x(j, split=False):
            xall[j] = xpool.tile([128, NDCH, TQ], BF16, tag="xall",
                                 name=f"x{j}")
            if split:
                nc.sync.dma_start(xall[j][:, 0:4], xsrc[:, 0:4, j * TQ:(j + 1) * TQ])
                nc.scalar.dma_start(xall[j][:, 4:8], xsrc[:, 4:8, j * TQ:(j + 1) * TQ])
            else:
                nc.sync.dma_start(xall[j][:], xsrc[:, :, j * TQ:(j + 1) * TQ])

        load_x(0, split=True)
        nc.gpsimd.dma_start(wqk_sb[:, :, 0:DL], wqk_src[:, :, 0:DL])
        nc.scalar.dma_start(wqk_sb[:, :, DL:2 * DL], wqk_src[:, :, DL:2 * DL])
        nc.scalar.dma_start(
            wv_sb[:], wv_d.rearrange("(dd p) m -> p dd m", p=128))
        nc.gpsimd.dma_start(bias_sb[:], bqk_d)
        if not vbias_zero:
            nc.gpsimd.dma_start(bv_f[:], bv_d)
            nc.vector.tensor_copy(bv_r[:], bv_f[:])
        if causal:
            nc.gpsimd.dma_start(
                maskt[:], maskt_d.rearrange("p (two m) -> p two m", two=2))
        nc.gpsimd.dma_start(
            wp_sb[:], wp_d.rearrange("(kk p) m -> p kk m", p=128))

        # ---------------- step generators ----------------
        def qkv_steps(j):
            """12 closures: 8 QK m-tile groups + 4 V chunk groups."""
            jsl = slice(j * TQ, (j + 1) * TQ)
            steps = []

            def qk_group(m, j=j, jsl=jsl):
                ps = ps_m.tile([128, TQ], F32, tag="psm", name=f"qk{j}_{m}")
                for d in range(NDCH):
                    nc.tensor.matmul(
                        ps[:], wqk_sb[:, d, m * 128:(m + 1) * 128],
                        xall[j][:, d, :], start=(d == 0), stop=(d == NDCH - 1))
                dest = qT[m] if m < 4 else kT[m - 4]
                nc.vector.tensor_scalar_add(dest[:, jsl], ps[:],
                                            bias_sb[:, m:m + 1])

            def v_group(c4, j=j):
                tt = 4 * j + c4
                ps = ps_m.tile([128, DL], F32, tag="psm", name=f"v{tt}")
                for d in range(NDCH):
                    nc.tensor.matmul(
                        ps[:], xall[j][:, d, c4 * 128:(c4 + 1) * 128],
                        wv_sb[:, d, :], start=(d == 0),
                        stop=(vbias_zero and d == NDCH - 1))
                if not vbias_zero:
                    nc.tensor.matmul(ps[:], ones_r[:, 0:128], bv_r[:],
                                     start=False, stop=True)
                nc.vector.tensor_copy(
                    vs[tt][:, :, 0:64],
                    ps[:].rearrange("p (h v) -> p h v", h=HL))

            for m in range(8):
                steps.append(lambda m=m: qk_group(m))
            for c4 in range(4):
                steps.append(lambda c4=c4: v_group(c4))
            return steps

        def proj_steps(j, pool=None):
            """8 closures: per t-tile (4) x per half (2)."""
            steps = []
            ot = [None, None, None, None]

            def pstep(t4, nb, j=j):
                t = 4 * j + t4
                if nb == 0:
                    ot[t4] = otpool.tile([128, D], F32, tag="ot",
                                         name=f"ot{t}")
                if pool is None:
                    ps = ps_m.tile([128, TQ], F32, tag="psm",
                                   name=f"p3_{t}_{nb}")
                else:
                    ps = pool.tile([128, TQ], F32, tag="po",
                                   name=f"p3_{t}_{nb}")
                for k in range(4):
                    nc.tensor.matmul(
                        ps[:], yT[k][:, t * 128:(t + 1) * 128],
                        wp_sb[:, k, nb * TQ:(nb + 1) * TQ],
                        start=(k == 0), stop=(k == 3))
                nc.vector.tensor_copy(ot[t4][:, nb * TQ:(nb + 1) * TQ], ps[:])
                if nb == 1:
                    eng = nc.gpsimd if t % 2 == 0 else nc.sync
                    eng.dma_start(out_d[t * 128:(t + 1) * 128, :], ot[t4][:])

            for t4 in range(4):
                for nb in range(2):
                    steps.append(lambda t4=t4, nb=nb: pstep(t4, nb))
            return steps

        # ---------------- fused pipeline ----------------
        pro = qkv_steps(0)            # prologue: QKV for block 0; emit the
        order = [0, 4, 8, 9, 10, 11, 1, 5, 2, 6, 3, 7]   # pair-0 deps first
        for ix in order:
            pro[ix]()

        for j in range(NQB):
            jsl = slice(j * TQ, (j + 1) * TQ)
            fillers = deque()
            # deep x prefetch: xall[j+1] must land well before its QKV steps
            # run (the PE's hoisted LDWEIGHTS can otherwise race the DMA).
            if j == 0:
                load_x(1)
                load_x(2)
            elif j + 2 < NQB:
                load_x(j + 2)
            if j + 1 < NQB:
                fillers.extend(qkv_steps(j + 1))
            if j >= 1:
                fillers.extend(proj_steps(j - 1))

            cs = list(range(4 * (j + 1))) if causal else list(range(NKC))
            for i in range(4):        # head pair (2i, 2i+1)
                poA = ps_o.tile([128, TQ], F32, tag="po", name=f"poA{j}_{i}")
                poB = ps_o.tile([128, TQ], F32, tag="po", name=f"poB{j}_{i}")

                pend = None           # PV(c) emitted after exp(c+1)
                for ci, c in enumerate(cs):
                    csl = slice(c * TKC, (c + 1) * TKC)
                    s = c - 4 * j if (causal and c >= 4 * j) else None
                    lo = s * TKC if s else 0
                    ss = ps_s.tile([TKC, 2, TQ], F32, tag="ss",
                                   name=f"ss{j}_{i}_{c}")
                    # two row-tiled K=64 matmuls -> run concurrently
                    nc.tensor.matmul(
                        ss[:, 0, lo:TQ], kT[i][0:64, csl],
                        qT[i][0:64, j * TQ + lo:(j + 1) * TQ],
                        start=True, stop=True)
                    nc.tensor.matmul(
                        ss[:, 1, lo:TQ], kT[i][64:128, csl],
                        qT[i][64:128, j * TQ + lo:(j + 1) * TQ],
                        start=True, stop=True)
                    pt = ppool.tile([TKC, 2, TQ], BF16, tag="pt",
                                    name=f"pt{j}_{i}_{c}")
                    nc.scalar.activation(pt[:, :, lo:TQ], ss[:, :, lo:TQ],
                                         EXP, scale=0.125)
                    if s is not None:
                        nc.vector.tensor_mul(pt[:, :, lo:lo + TKC],
                                             pt[:, :, lo:lo + TKC], maskt[:])
                    if pend is not None:
                        pc, ppt, plo = pend
                        st = (ci == 1)
                        nc.tensor.matmul(
                            poA[:, plo:TQ], vs[pc][:, 2 * i, :],
                            ppt[:, 0, plo:TQ], start=st, stop=False)
                        nc.tensor.matmul(
                            poB[:, plo:TQ], vs[pc][:, 2 * i + 1, :],
                            ppt[:, 1, plo:TQ], start=st, stop=False)
                    if fillers:
                        fillers.popleft()()
                    pend = (c, pt, lo)

                pc, ppt, plo = pend
                one = (len(cs) == 1)
                nc.tensor.matmul(poA[:, plo:TQ], vs[pc][:, 2 * i, :],
                                 ppt[:, 0, plo:TQ], start=one, stop=True)
                nc.tensor.matmul(poB[:, plo:TQ], vs[pc][:, 2 * i + 1, :],
                                 ppt[:, 1, plo:TQ], start=one, stop=True)

                # normalize: rows 64:127 of po already hold the broadcast
                # denominator; copy to SBUF, reciprocal, multiply.  Every DVE
                # op keeps its operands' base partitions aligned (mismatched
                # input bases in one DVE op silently read wrong partitions).
                dsb = rcpool.tile([64, 2 * TQ], F32, tag="dsb", name=f"ds{j}_{i}")
                rc = rcpool.tile([64, 2 * TQ], F32, tag="rc", name=f"rc{j}_{i}")
                nc.vector.tensor_copy(dsb[:, 0:TQ], poA[64:128, :])
                nc.vector.tensor_copy(dsb[:, TQ:2 * TQ], poB[64:128, :])
                nc.vector.reciprocal_approx_fast(out=rc[:], in_=dsb[:])
                nc.vector.tensor_mul(yT[i][0:64, jsl], poA[0:64, :],
                                     rc[:, 0:TQ])
                nc.vector.tensor_mul(yT[i][64:128, jsl], poB[0:64, :],
                                     rc[:, TQ:2 * TQ])

            while fillers:            # flush leftover interleaved steps
                fillers.popleft()()

        for step in proj_steps(NQB - 1, pool=ps_o):   # epilogue
            step()

    nc.compile()
    return nc


def _get_nc(causal: bool, vbias_zero: bool = True):
    key = (causal, vbias_zero)
    if key not in _CACHE:
        _CACHE[key] = _build(causal, vbias_zero)
    return _CACHE[key]


def _host_mask_tri() -> np.ndarray:
    i = np.arange(TKC)[:, None]
    jj = np.arange(TKC)[None, :]
    tri = (jj >= i).astype(np.float32)
    return np.ascontiguousarray(
        np.concatenate([tri, tri], axis=1).astype(ml_dtypes.bfloat16))


def _make_in_maps(x, W_qkv, b_qkv, W_proj):
    mask_np = _host_mask_tri()
    bf = ml_dtypes.bfloat16
    in_maps = []
    for core in range(N_CORES):
        b, g = core // 2, core % 2
        qc = slice(g * DL, (g + 1) * DL)
        kc = slice(D + g * DL, D + (g + 1) * DL)
        vc = slice(2 * D + g * DL, 2 * D + (g + 1) * DL)
        in_maps.append({
            "xT": np.ascontiguousarray(x[b].T.astype(bf)),
            "wqk": np.ascontiguousarray(np.concatenate(
                [W_qkv[:, qc], W_qkv[:, kc]], axis=1).astype(bf)),
            "wv": np.ascontiguousarray(W_qkv[:, vc].astype(bf)),
            "bqk": np.ascontiguousarray(
                np.concatenate([b_qkv[qc], b_qkv[kc]]).reshape(8, 128).T),
            "bv": np.ascontiguousarray(b_qkv[vc].reshape(1, DL)),
            "wproj": np.ascontiguousarray(
                W_proj[g * DL:(g + 1) * DL, :].astype(bf)),
            "maskt": mask_np,
        })
    return in_maps


def kernel(x, mask, W_qkv, b_qkv, W_proj, b_proj):
    x = np.asarray(x, dtype=np.float32)
    mask2d = np.asarray(mask, dtype=np.int32).reshape(T, T)
    W_qkv = np.asarray(W_qkv, dtype=np.float32)
    b_qkv = np.asarray(b_qkv, dtype=np.float32)
    W_proj = np.asarray(W_proj, dtype=np.float32)
    b_proj = np.asarray(b_proj, dtype=np.float32)

    if np.array_equal(mask2d, np.tril(np.ones((T, T), dtype=np.int32))):
        causal = True
    elif np.all(mask2d == 1):
        causal = False
    else:
        raise NotImplementedError("only causal (tril) or all-ones masks")

    vz = not np.any(b_qkv[2 * D:])
    nc = _get_nc(causal, vz)
    in_maps = _make_in_maps(x, W_qkv, b_qkv, W_proj)
    res = run_bass_kernel_spmd(nc, in_maps, core_ids=list(range(N_CORES)))
    out = np.empty((B, T, D), dtype=np.float32)
    for b in range(B):
        out[b] = (res.results[2 * b]["out"] + res.results[2 * b + 1]["out"]
                  + b_proj[None, :])
    return out
